# revision 33
# baseline (speedup 1.0000x reference)
"""Trainium2 Bass kernel for nn_MidmLMHeadModelWrapper (dense transformer
attention layer: QKV proj + partial RoPE + KV-cache update + softmax
attention + output projection), tensor-parallel over heads on 8 NeuronCores.

Sharding: heads 4c..4c+3 on core c.  QKV weight columns and proj weight rows
shard with heads; the KV cache shards with heads; the final projection is a
row-sharded matmul whose partial outputs are summed on the host (the unshard
step).

Two device programs:

* fast path (attention_mask == 0 and b_qkv == 0, which holds for this
  problem's inputs): everything in bf16 with fp32 PSUM accumulation.  The
  hidden state is loaded once (bf16) and reused for Q, K and V projections;
  RoPE runs PSUM-free (SBUF->SBUF DMA partition shift + sign folded into a
  host-precomputed sin, ALPHA folded into the Q weights); softmax skips the
  mask add, exp reads scores straight from PSUM, and the denominator
  accumulates on the vector engine instead of a PE matmul.  The attention
  inner loop is software-pipelined so the PE never waits on exp.
* generic path (any mask/bias): the original f32r-scores implementation.
"""

import numpy as np
import ml_dtypes
from contextlib import ExitStack

import concourse.bass as bass
import concourse.tile as tile
from concourse import mybir
from concourse.bass_utils import run_bass_kernel_spmd

# problem shapes (hardcoded per contract)
B, Q, D = 2, 512, 4096
H, HD = 32, 128
MAXLEN, ROT = 4096, 64
N_CORES = 8
HL = H // N_CORES          # 4 heads per core
BQ = B * Q                 # 1024
NKT = D // 128             # 32 contraction tiles over D
NST = MAXLEN // 128        # 32 seq tiles over the cache
ALPHA = 1.0 / float(HD) ** 0.5

f32 = mybir.dt.float32
f32r = mybir.dt.float32r
bf16 = mybir.dt.bfloat16
AF = mybir.ActivationFunctionType


def _split_multi_waits(nc, max_waits=1):
    """This container's walrus supports ONE inline sync-wait per instruction.
    Move excess waits onto standalone EventSemaphore instructions inserted
    immediately before, preserving per-engine program order."""
    ctr = 0
    for f in nc.m.functions:
        for bb in f.blocks:
            changed = False
            new_insts = []
            for inst in bb.instructions:
                si = inst.sync_info
                if si is not None and len(si.on_wait) > max_waits:
                    waits = list(si.on_wait)
                    extra, keep = waits[:-max_waits], waits[-max_waits:]
                    for w in extra:
                        ctr += 1
                        ev = mybir.InstEventSemaphore(
                            name=f"I-waitsplit-{ctr}", ins=[], outs=[])
                        ev.engine = inst.engine
                        ev.sync_info = mybir.SyncInfo(on_wait=[w], on_update=[])
                        new_insts.append(ev)
                    si.on_wait = keep
                    inst.sync_info = si
                    changed = True
                new_insts.append(inst)
            if changed:
                bb.instructions = new_insts
    return ctr


# ===================== fast path (zero mask, zero qkv bias) ==================

def build_program_fast(step_tile: int, repeats: int = 1):
    nc = bass.Bass()

    hb = nc.dram_tensor("hb", [D, BQ], bf16, kind="ExternalInput")
    wq = nc.dram_tensor("wq", [D, 512], bf16, kind="ExternalInput")
    wk = nc.dram_tensor("wk", [D, 512], bf16, kind="ExternalInput")
    wv = nc.dram_tensor("wv", [D, 512], bf16, kind="ExternalInput")
    cosb = nc.dram_tensor("cosb", [ROT, BQ], bf16, kind="ExternalInput")
    sinmb = nc.dram_tensor("sinmb", [ROT, BQ], bf16, kind="ExternalInput")
    kTp = nc.dram_tensor("kTp", [B * HL, 128, MAXLEN], bf16, kind="ExternalInput")
    vsw = nc.dram_tensor("vsw", [B * HL, 128, NST * HD], bf16, kind="ExternalInput")
    wp = nc.dram_tensor("wp", [HL * HD, D], bf16, kind="ExternalInput")
    onesb = nc.dram_tensor("onesb", [128, 1], bf16, kind="ExternalInput")
    onesr = nc.dram_tensor("onesr", [1, 128], f32r, kind="ExternalInput")
    outp = nc.dram_tensor("outp", [BQ, D], bf16, kind="ExternalOutput")

    with tile.TileContext(nc) as tc:
        with ExitStack() as octx:
            persist = octx.enter_context(tc.tile_pool(name="persist", bufs=1))
            consts = octx.enter_context(tc.tile_pool(name="consts", bufs=1))

            onesr_t = consts.tile([1, 128], f32r, tag="onesr")
            nc.gpsimd.dma_start(onesr_t[:], onesr[:])
            onesb_t = consts.tile([128, 1], bf16, tag="onesb")
            nc.gpsimd.dma_start(onesb_t[:], onesb[:])

            for rep in range(repeats):
                _emit_fast(nc, tc, persist, onesr_t, onesb_t,
                           hb, wq, wk, wv, cosb, sinmb, kTp, vsw, wp, outp,
                           step_tile)

    _split_multi_waits(nc)
    return nc


def _emit_fast(nc, tc, persist, onesr_t, onesb_t,
               hb, wq, wk, wv, cosb, sinmb, kTp, vsw, wp, outp, step_tile):
    new_lo, new_hi = step_tile, step_tile + Q // 128

    qkT = {}    # (s, b) -> [128, Q] bf16 ; s 0..3 = q heads, 4..7 = k heads
    v_new = {}  # st 0..7 -> [128, 512] bf16

    # phase-3 slab pools opened early so bh 0/1 prefetch during phase 1
    kpool_ctx = tc.tile_pool(name="katt", bufs=2)
    kpool = kpool_ctx.__enter__()
    vpool_ctx = tc.tile_pool(name="vatt", bufs=2)
    vpool = vpool_ctx.__enter__()

    # ---------------- phase 1: projections + RoPE -----------------
    with tc.tile_pool(name="ph1h", bufs=1) as hpool, \
         tc.tile_pool(name="ph1c", bufs=1) as cpool, \
         tc.tile_pool(name="ph1w", bufs=4) as wpool, \
         tc.tile_pool(name="ph1r", bufs=8) as rpool:

        # bf16 hidden, resident for all three projection passes.  One slab,
        # 8 chunked 1-MiB DMAs on the sync queue: big transfers fan out
        # across all 16 SDMA engines (~340 GB/s); many small ones serialize
        # on per-transfer setup.  Weights stream as 2-MiB half-pass slabs on
        # the gpsimd queue.
        hslab = hpool.tile([128, NKT * BQ], bf16, tag="hslab")
        hbview = hb[:].rearrange("(t p) c -> p t c", p=128)
        hsview = hslab[:].rearrange("p (t c) -> p t c", c=BQ)
        for j in range(8):
            nc.scalar.dma_start(hsview[:, j * 4:(j + 1) * 4, :],
                                hbview[:, j * 4:(j + 1) * 4, :])

        def h_mv(kt, c0, c1):
            return hslab[:, kt * BQ + c0: kt * BQ + c1]

        cosw = cpool.tile([ROT, BQ], bf16, tag="cosw")
        nc.scalar.dma_start(cosw[:], cosb[:])
        sinw = cpool.tile([ROT, BQ], bf16, tag="sinw")
        nc.scalar.dma_start(sinw[:], sinmb[:])

        HKT = NKT // 4   # kt tiles per weight quarter-slab

        def load_wquarter(wsrc, quarter, split=False):
            wt = wpool.tile([128, HKT * 512], bf16, tag="wsl")
            wview = wsrc[:].rearrange("(t p) c -> p t c", p=128)
            wtv = wt[:].rearrange("p (t c) -> p t c", c=512)
            q0 = quarter * HKT
            if split:
                nc.sync.dma_start(wtv[:, 0:HKT // 2, :],
                                  wview[:, q0:q0 + HKT // 2, :])
                nc.sync.dma_start(wtv[:, HKT // 2:HKT, :],
                                  wview[:, q0 + HKT // 2:q0 + HKT, :])
            else:
                nc.sync.dma_start(wtv[:], wview[:, q0:q0 + HKT, :])
            return wt

        # prefetch the first two attention bh's k/v slabs during phase 1
        prefetched = {}
        for pbh in range(2):
            kslab = kpool.tile([128, MAXLEN], bf16, tag="kslab",
                               name=f"kpre{pbh}")
            lo, hi = new_lo * 128, new_hi * 128
            nc.gpsimd.dma_start(kslab[:, 0:lo], kTp[pbh, :, 0:lo])
            nc.gpsimd.dma_start(kslab[:, hi:MAXLEN], kTp[pbh, :, hi:MAXLEN])
            vslab = vpool.tile([128, NST * HD], bf16, tag="vslab",
                               name=f"vpre{pbh}")
            nc.gpsimd.dma_start(vslab[:, 0:new_lo * HD],
                                vsw[pbh, :, 0:new_lo * HD])
            nc.gpsimd.dma_start(vslab[:, new_hi * HD:NST * HD],
                                vsw[pbh, :, new_hi * HD:NST * HD])
            prefetched[pbh] = (kslab, vslab)

        for pq, (wsrc, srange) in enumerate(((wq, range(0, 4)),
                                             (wk, range(4, 8)))):
            with tc.tile_pool(name=f"qkps{pq}", bufs=1, space="PSUM") as qkps:
                psums = {}
                for s in srange:
                    psums[s] = qkps.tile([128, BQ], f32, tag=f"qk{s % 4}",
                                         name=f"qkps{s}")
                for quarter in range(4):
                    wt = load_wquarter(wsrc, quarter,
                                       split=(pq == 0 and quarter == 0))
                    for kt in range(quarter * HKT, (quarter + 1) * HKT):
                        woff = (kt - quarter * HKT) * 512
                        for s in srange:
                            for nh in range(2):
                                nc.tensor.matmul(
                                    psums[s][:, nh * 512:(nh + 1) * 512],
                                    wt[:, woff + (s % 4) * 128:
                                       woff + (s % 4 + 1) * 128],
                                    h_mv(kt, nh * 512, (nh + 1) * 512),
                                    start=(kt == 0), stop=(kt == NKT - 1))
                raw = {}
                for s in srange:
                    ft = rpool.tile([128, BQ], bf16, tag="ftmp")
                    if s % 2:
                        nc.scalar.copy(ft[:], psums[s][:])
                    else:
                        nc.vector.tensor_copy(ft[:], psums[s][:])
                    raw[s] = ft
            # PSUM-free RoPE into a fresh wide qf tile: the plain ft copy
            # above is the ONLY psum reader, so the psum pool releases after
            # 4 wide copies and the next pass's matmuls start immediately;
            # rope and the passthrough copy read ft from SBUF off the
            # critical path.
            for s in srange:
                ft = raw[s]
                qf = persist.tile([128, BQ], bf16, tag=f"qkTw_{s}")
                sh = rpool.tile([ROT, BQ], bf16, tag="sh")
                nc.scalar.dma_start(sh[0:ROT // 2, :], ft[ROT // 2:ROT, :])
                nc.scalar.dma_start(sh[ROT // 2:ROT, :], ft[0:ROT // 2, :])
                nc.vector.tensor_mul(qf[0:ROT, :], ft[0:ROT, :], cosw[:])
                nc.vector.tensor_mul(sh[:], sh[:], sinw[:])
                nc.vector.tensor_add(qf[0:ROT, :], qf[0:ROT, :], sh[:])
                nc.scalar.copy(qf[ROT:128, :], ft[ROT:128, :])
                qkT[s] = qf

        with tc.tile_pool(name="vps", bufs=1, space="PSUM") as vps:
            vpsums = {}
            for g in range(4):
                vpsums[g] = vps.tile([128, BQ], f32, tag=f"v{g}",
                                     name=f"vpsw{g}")
            for quarter in range(4):
                wt = load_wquarter(wv, quarter)
                for kt in range(quarter * HKT, (quarter + 1) * HKT):
                    woff = (kt - quarter * HKT) * 512
                    for st in range(8):
                        nc.tensor.matmul(
                            vpsums[st // 2][:, (st % 2) * 512:
                                            (st % 2 + 1) * 512],
                            h_mv(kt, st * 128, (st + 1) * 128),
                            wt[:, woff:woff + 512],
                            start=(kt == 0), stop=(kt == NKT - 1))
            for g in range(4):
                vt = persist.tile([128, BQ], bf16, tag=f"vneww_{g}")
                if g % 2:
                    nc.scalar.copy(vt[:], vpsums[g][:])
                else:
                    nc.vector.tensor_copy(vt[:], vpsums[g][:])
                v_new[g] = vt

    # ---------------- phase 3: attention per (b, h) -----------------
    outT = {}  # bh -> [128 hd, 512 q] bf16
    wp4ctx = tc.tile_pool(name="wp4", bufs=1)
    wp4 = wp4ctx.__enter__()
    oevctx = tc.tile_pool(name="oev4", bufs=2)
    oevp = oevctx.__enter__()
    oev_slabs = {}
    with tc.tile_pool(name="aacc", bufs=2) as apool, \
         tc.tile_pool(name="eatt", bufs=3) as epool, \
         tc.tile_pool(name="datt", bufs=2) as dpool, \
         tc.tile_pool(name="spsp", bufs=2, space="PSUM") as scps, \
         tc.tile_pool(name="apsum", bufs=2, space="PSUM") as aps, \
         tc.tile_pool(name="dpsum", bufs=1, space="PSUM") as dps_pool, \
         tc.tile_pool(name="bpsum", bufs=1, space="PSUM") as bps_pool:

        # phase-4 groups (4 matmuls + copy + final store) emitted either
        # interleaved into late-phase-3 PE slack (psum from the shared
        # bps bank pool, copy on DVE) or in the phase-4 tail (pps4, ACT)
        def emit_ph4_group(b4, st4, ncn, psum_pool, use_dve):
            if (b4, st4) not in oev_slabs:
                oev_t = oevp.tile([128, D], bf16, tag="oev",
                                  name=f"oev{b4}_{st4}")
                oev_slabs[(b4, st4)] = oev_t
            oev = oev_slabs[(b4, st4)]
            ops = psum_pool.tile([128, 512], f32, tag="bps" if use_dve else "ops")
            for kt4 in range(HL):
                nc.tensor.matmul(
                    ops[:],
                    outT[b4 * 4 + kt4][:, st4 * 128:(st4 + 1) * 128],
                    wpt[kt4][:, ncn * 512:(ncn + 1) * 512],
                    start=(kt4 == 0), stop=(kt4 == HL - 1))
            if use_dve:
                nc.vector.tensor_copy(oev[:, ncn * 512:(ncn + 1) * 512], ops[:])
            else:
                nc.scalar.copy(oev[:, ncn * 512:(ncn + 1) * 512], ops[:])
            if ncn == 7:
                (nc.sync if (b4 * 4 + st4) % 2 else nc.gpsimd).dma_start(
                    outp[b4 * 512 + st4 * 128: b4 * 512 + (st4 + 1) * 128, :],
                    oev[:])
                del oev_slabs[(b4, st4)]

        ph4q = []

        # phase-4 weights prefetch on the scalar queue through phase 3
        wpt = {}
        for kt in range(HL):
            wpt[kt] = wp4.tile([128, D], bf16, tag=f"wp{kt}", name=f"wpt{kt}")
            nc.scalar.dma_start(wpt[kt][:], wp[kt * 128:(kt + 1) * 128, :])

        pend_a, pend_b = [], []   # deferred normalize halves of previous bh

        for bh in range(B * HL):
            b, h = divmod(bh, HL)
            if bh in prefetched:
                kslab, vslab = prefetched.pop(bh)
            else:
                kslab = kpool.tile([128, MAXLEN], bf16, tag="kslab")
                lo, hi = new_lo * 128, new_hi * 128
                nc.sync.dma_start(kslab[:, 0:lo], kTp[bh, :, 0:lo])
                nc.sync.dma_start(kslab[:, hi:MAXLEN], kTp[bh, :, hi:MAXLEN])
                vslab = vpool.tile([128, NST * HD], bf16, tag="vslab")
                nc.gpsimd.dma_start(vslab[:, 0:new_lo * HD],
                                    vsw[bh, :, 0:new_lo * HD])
                nc.gpsimd.dma_start(vslab[:, new_hi * HD:NST * HD],
                                    vsw[bh, :, new_hi * HD:NST * HD])

            acc = apool.tile([128, Q], bf16, tag="acc")
            accv = aps.tile([128, Q], f32, tag="accv")
            qt = qkT[h][:, b * 512:(b + 1) * 512]
            kqw = qkT[4 + h]

            def k_lhsT(kt, kslab=kslab, kqw=kqw, b=b):
                if new_lo <= kt < new_hi:
                    return kqw[:, b * 512 + (kt - new_lo) * 128:
                               b * 512 + (kt - new_lo + 1) * 128]
                return kslab[:, kt * 128:(kt + 1) * 128]

            def v_lhsT(kt, vslab=vslab, b=b, h=h):
                if new_lo <= kt < new_hi:
                    st = b * 4 + (kt - new_lo)
                    return v_new[st // 2][:, (st % 2) * 512 + h * HD:
                                          (st % 2) * 512 + (h + 1) * HD]
                return vslab[:, kt * HD:(kt + 1) * HD]

            NG = NST // 2       # groups of 2 kv tiles; one wide exp per group
            sps_t = [None] * NG

            def emit_tail(g, acc=acc, accv=accv, sps_t=sps_t,
                          v_lhsT=v_lhsT):
                ebf = epool.tile([128, 2 * Q], bf16, tag="ebf")
                nc.scalar.activation(ebf[:], sps_t[g][:], AF.Exp)
                for i in range(2):
                    j = 2 * g + i
                    eb = ebf[:, i * Q:(i + 1) * Q]
                    if j == 0:
                        nc.vector.tensor_copy(acc[:], eb)
                    else:
                        nc.vector.tensor_add(acc[:], acc[:], eb)
                    nc.tensor.matmul(accv[:], v_lhsT(j), eb,
                                     start=(j == 0), stop=(j == NST - 1))

            # software-pipelined by kv-tile pairs: PE stream is
            # sc0 sc1 | sc2 sc3 | av0 av1 sc4 sc5 | av2 av3 ... with one
            # [128,1024] exp per pair; prev bh's normalize at g==1/g==3
            for g in range(NG):
                sps = scps.tile([128, 2 * Q], f32, tag="sps")
                for i in range(2):
                    nc.tensor.matmul(sps[:, i * Q:(i + 1) * Q],
                                     k_lhsT(2 * g + i), qt,
                                     start=True, stop=True)
                sps_t[g] = sps
                if g == 1:
                    while pend_a:
                        pend_a.pop(0)()
                if g == 5:
                    while pend_b:
                        pend_b.pop(0)()
                if g >= 1:
                    emit_tail(g - 1)
                    if bh >= 5 and ph4q:
                        b4, st4, ncn = ph4q.pop(0)
                        emit_ph4_group(b4, st4, ncn, bps_pool, True)
            emit_tail(NG - 1)
            if bh == 4:
                ph4q.extend((0, st4, ncn) for st4 in range(4)
                            for ncn in range(8))

            # denominator + normalize, deferred into the next bh's pipeline
            state = {}

            def norm_a(bh=bh, acc=acc, state=state):
                dps = dps_pool.tile([1, Q], f32, tag="dps")
                nc.tensor.matmul(dps[:], onesb_t[:], acc[:],
                                 start=True, stop=True)
                rec = dpool.tile([1, Q], f32, tag="rec")
                nc.vector.reciprocal(rec[:], dps[:])
                rec_r = dpool.tile([1, Q], f32r, tag="recr")
                nc.vector.tensor_copy(rec_r[:], rec[:])
                state["rec_r"] = rec_r

            def norm_b(bh=bh, accv=accv, state=state):
                bps = bps_pool.tile([128, Q], f32, tag="bps")
                nc.tensor.matmul(bps[:], onesr_t[:], state["rec_r"][:],
                                 start=True, stop=True)
                bcs = dpool.tile([128, Q], f32, tag="bcs")
                nc.vector.tensor_copy(bcs[:], bps[:])
                ot = persist.tile([128, Q], bf16, tag=f"outT_{bh}")
                nc.vector.tensor_mul(ot[:], bcs[:], accv[:])
                outT[bh] = ot

            pend_a.append(norm_a)
            pend_b.append(norm_b)

        while pend_a:
            pend_a.pop(0)()
        while pend_b:
            pend_b.pop(0)()

    # leftover interleaved-queue entries plus all of b=1 (phase-3 psum freed)
    with tc.tile_pool(name="pps4", bufs=3, space="PSUM") as pps4:
        for b4, st4, ncn in ph4q:
            emit_ph4_group(b4, st4, ncn, pps4, False)
        for st4 in range(4):
            for ncn in range(8):
                emit_ph4_group(1, st4, ncn, pps4, False)
    oevctx.__exit__(None, None, None)
    wp4ctx.__exit__(None, None, None)
    vpool_ctx.__exit__(None, None, None)
    kpool_ctx.__exit__(None, None, None)
    return qkT, v_new, outT


# ===================== generic path (any mask / bias) ========================

def build_program_generic(step_tile: int, repeats: int = 1):
    """Emit the per-core program.  step_tile = current_step // 128."""
    nc = bass.Bass()

    hT = nc.dram_tensor("hT", [D, BQ], f32r, kind="ExternalInput")
    hTb = nc.dram_tensor("hTb", [D, BQ], bf16, kind="ExternalInput")
    wqk = nc.dram_tensor("wqk", [D, 8 * 128], f32r, kind="ExternalInput")
    wv = nc.dram_tensor("wv", [D, 512], bf16, kind="ExternalInput")
    bqk = nc.dram_tensor("bqk", [1, 8 * 128], f32r, kind="ExternalInput")
    bvb = nc.dram_tensor("bvb", [1, 512], bf16, kind="ExternalInput")
    ones = nc.dram_tensor("ones", [1, BQ], f32r, kind="ExternalInput")
    onesr = nc.dram_tensor("onesr", [1, 128], f32r, kind="ExternalInput")
    onesb = nc.dram_tensor("onesb", [128, 1], bf16, kind="ExternalInput")
    onesbr = nc.dram_tensor("onesbr", [1, 128], bf16, kind="ExternalInput")
    rmat = nc.dram_tensor("rmat", [ROT, ROT], f32, kind="ExternalInput")
    kTp = nc.dram_tensor("kTp", [B * HL, 128, MAXLEN], f32r, kind="ExternalInput")
    vp = nc.dram_tensor("vp", [B * HL, MAXLEN, HD], bf16, kind="ExternalInput")
    maskT = nc.dram_tensor("maskT", [B, MAXLEN, Q], f32, kind="ExternalInput")
    cosT = nc.dram_tensor("cosT", [B, ROT, Q], f32, kind="ExternalInput")
    sinT = nc.dram_tensor("sinT", [B, ROT, Q], f32, kind="ExternalInput")
    wp = nc.dram_tensor("wp", [HL * HD, D], bf16, kind="ExternalInput")
    outp = nc.dram_tensor("outp", [BQ, D], f32, kind="ExternalOutput")

    with tile.TileContext(nc) as tc:
        with ExitStack() as octx:
            persist = octx.enter_context(tc.tile_pool(name="persist", bufs=1))
            consts = octx.enter_context(tc.tile_pool(name="consts", bufs=1))

            onesr_t = consts.tile([1, 128], f32r, tag="onesr")
            nc.gpsimd.dma_start(onesr_t[:], onesr[:])
            onesb_t = consts.tile([128, 1], bf16, tag="onesb")
            nc.gpsimd.dma_start(onesb_t[:], onesb[:])

            for rep in range(repeats):
                _emit_generic(nc, tc, persist, onesr_t, onesb_t,
                              hT, hTb, wqk, wv, bqk, bvb, ones, onesbr, rmat,
                              kTp, vp, maskT, cosT, sinT, wp, outp, step_tile)

    _split_multi_waits(nc)
    return nc


def _emit_generic(nc, tc, persist, onesr_t, onesb_t,
                  hT, hTb, wqk, wv, bqk, bvb, ones, onesbr, rmat,
                  kTp, vp, maskT, cosT, sinT, wp, outp, step_tile):
    new_lo, new_hi = step_tile, step_tile + Q // 128

    qkT = {}    # (s, b) -> [128, Q] f32r ; s 0..3 = q heads, 4..7 = k heads
    v_new = {}  # st 0..7 -> [128, 512] bf16

    # ---------------- phase 1: projections -----------------
    with tc.tile_pool(name="ph1c", bufs=1) as cpool, \
         tc.tile_pool(name="ph1h", bufs=5) as hpool, \
         tc.tile_pool(name="ph1w", bufs=4) as wpool, \
         tc.tile_pool(name="ph1t", bufs=1) as tpool, \
         tc.tile_pool(name="ph1e", bufs=3) as epool:

        ones_t = cpool.tile([1, BQ], f32r, tag="ones")
        nc.gpsimd.dma_start(ones_t[:], ones[:])
        onesbr_t = cpool.tile([1, 128], bf16, tag="onesbr")
        nc.gpsimd.dma_start(onesbr_t[:], onesbr[:])
        rmat_t = cpool.tile([ROT, ROT], f32, tag="rmat")
        nc.gpsimd.dma_start(rmat_t[:], rmat[:])
        bqk_t = cpool.tile([1, 8 * 128], f32r, tag="bqk")
        nc.gpsimd.dma_start(bqk_t[:], bqk[:])
        bvb_t = cpool.tile([1, 512], bf16, tag="bvb")
        nc.gpsimd.dma_start(bvb_t[:], bvb[:])
        cos_t, sin_t = {}, {}
        for b in range(B):
            cos_t[b] = cpool.tile([ROT, Q], f32, tag=f"cos{b}", name=f"cos{b}")
            nc.gpsimd.dma_start(cos_t[b][:], cosT[b])
            sin_t[b] = cpool.tile([ROT, Q], f32, tag=f"sin{b}", name=f"sin{b}")
            nc.gpsimd.dma_start(sin_t[b][:], sinT[b])

        # bf16 resident hidden for the V pass: slab[p, kt*BQ + c]
        hslab = cpool.tile([128, NKT * BQ], bf16, tag="hslab")
        hbview = hTb[:].rearrange("(t p) c -> p t c", p=128)
        for j in range(8):
            (nc.sync if j % 2 else nc.gpsimd).dma_start(
                hslab[:].rearrange("p (t c) -> p t c", c=BQ)[:, j * 4:(j + 1) * 4, :],
                hbview[:, j * 4:(j + 1) * 4, :])

        # --- passes Q (s 0..3) and K (s 4..7): f32r, streamed hT ---
        for pq, srange in ((0, range(0, 4)), (1, range(4, 8))):
            t32 = {}
            with tc.tile_pool(name=f"qkps{pq}", bufs=1, space="PSUM") as qkps:
                psums = {}
                for s in srange:
                    psums[s] = qkps.tile([128, BQ], f32, tag=f"qk{s % 4}", name=f"qkps{s}")
                for kt in range(NKT):
                    ht = hpool.tile([128, BQ], f32r, tag="ht")
                    eng = nc.sync if kt % 2 == 0 else nc.gpsimd
                    eng2 = nc.gpsimd if kt % 2 == 0 else nc.sync
                    eng.dma_start(ht[:, 0:512], hT[kt * 128:(kt + 1) * 128, 0:512])
                    eng2.dma_start(ht[:, 512:BQ], hT[kt * 128:(kt + 1) * 128, 512:BQ])
                    wt = wpool.tile([128, 512], f32r, tag="wt")
                    eng2.dma_start(wt[:], wqk[kt * 128:(kt + 1) * 128,
                                              pq * 512:(pq + 1) * 512])
                    for s in srange:
                        for nh in range(2):
                            nc.tensor.matmul(
                                psums[s][:, nh * 512:(nh + 1) * 512],
                                wt[:, (s % 4) * 128:(s % 4 + 1) * 128],
                                ht[:, nh * 512:(nh + 1) * 512],
                                start=(kt == 0), stop=False)
                for s in srange:
                    for nh in range(2):
                        nc.tensor.matmul(
                            psums[s][:, nh * 512:(nh + 1) * 512],
                            bqk_t[0:1, s * 128:(s + 1) * 128],
                            ones_t[0:1, nh * 512:(nh + 1) * 512],
                            start=False, stop=True)
                for s in srange:
                    for b in range(B):
                        t = tpool.tile([128, Q], f32, tag=f"t32_{s % 4}_{b}")
                        half = psums[s][:, b * 512:(b + 1) * 512]
                        if s < 4:
                            nc.scalar.mul(t[:], half, ALPHA)
                        else:
                            nc.scalar.copy(t[:], half)
                        t32[(s, b)] = t
            # RoPE (qk psums released; partner psums now fit)
            with tc.tile_pool(name=f"rope{pq}", bufs=2, space="PSUM") as rps:
                for s in srange:
                    for b in range(B):
                        t = t32[(s, b)]
                        pps = rps.tile([ROT, Q], f32, tag="pps")
                        nc.tensor.matmul(pps[:], rmat_t[:], t[0:ROT, :],
                                         start=True, stop=True)
                        ps_sin = epool.tile([ROT, Q], f32, tag="psin")
                        nc.vector.tensor_mul(ps_sin[:], sin_t[b][:], pps[:])
                        nc.vector.tensor_mul(t[0:ROT, :], t[0:ROT, :],
                                             cos_t[b][:])
                        nc.vector.tensor_add(t[0:ROT, :], t[0:ROT, :],
                                             ps_sin[:])
                        ft = persist.tile([128, Q], f32r, tag=f"qkT_{s}_{b}")
                        nc.vector.tensor_copy(ft[:], t[:])
                        qkT[(s, b)] = ft

        # --- pass V: bf16, resident hidden ---
        with tc.tile_pool(name="vps", bufs=1, space="PSUM") as vps:
            vpsums = {}
            for st in range(8):
                vpsums[st] = vps.tile([128, 512], f32, tag=f"v{st}", name=f"vps{st}")
            for kt in range(NKT):
                wt = wpool.tile([128, 512], bf16, tag="wtv")
                (nc.sync if kt % 2 else nc.gpsimd).dma_start(
                    wt[:], wv[kt * 128:(kt + 1) * 128, :])
                for st in range(8):
                    nc.tensor.matmul(
                        vpsums[st][:],
                        hslab[:, kt * BQ + st * 128: kt * BQ + (st + 1) * 128],
                        wt[:], start=(kt == 0), stop=False)
            for st in range(8):
                nc.tensor.matmul(vpsums[st][:], onesbr_t[:], bvb_t[:],
                                 start=False, stop=True)
                vt = persist.tile([128, 512], bf16, tag=f"vnew_{st}")
                nc.vector.tensor_copy(vt[:], vpsums[st][:])
                v_new[st] = vt

    # ---------------- phase 3: attention per (b, h) -----------------
    outT = {}  # bh -> [128 hd, 512 q] bf16
    with tc.tile_pool(name="katt", bufs=2) as kpool, \
         tc.tile_pool(name="vatt", bufs=2) as vpool, \
         tc.tile_pool(name="matt", bufs=1) as mpool, \
         tc.tile_pool(name="eatt", bufs=5) as epool, \
         tc.tile_pool(name="datt", bufs=2) as dpool, \
         tc.tile_pool(name="spsp", bufs=4, space="PSUM") as scps, \
         tc.tile_pool(name="apsum", bufs=2, space="PSUM") as aps, \
         tc.tile_pool(name="dpsum", bufs=1, space="PSUM") as dps_pool, \
         tc.tile_pool(name="bpsum", bufs=1, space="PSUM") as bps_pool:

        mask_slab = None
        for bh in range(B * HL):
            b, h = divmod(bh, HL)
            if h == 0:
                mask_slab = mpool.tile([128, NST * Q], f32, tag="mask")
                mview = maskT[b].rearrange("(t p) q -> p t q", p=128)
                for j in range(8):
                    (nc.sync if j % 2 else nc.gpsimd).dma_start(
                        mask_slab[:].rearrange("p (t q) -> p t q", q=Q)
                        [:, j * 4:(j + 1) * 4, :],
                        mview[:, j * 4:(j + 1) * 4, :])

            kslab = kpool.tile([128, MAXLEN], f32r, tag="kslab")
            lo, hi = new_lo * 128, new_hi * 128
            for j, (a0, a1) in enumerate(
                    [(0, lo // 2), (lo // 2, lo),
                     (hi, (hi + MAXLEN) // 2), ((hi + MAXLEN) // 2, MAXLEN)]):
                m = (a0 + a1) // 2
                (nc.sync if j % 2 else nc.gpsimd).dma_start(
                    kslab[:, a0:m], kTp[bh, :, a0:m])
                (nc.gpsimd if j % 2 else nc.sync).dma_start(
                    kslab[:, m:a1], kTp[bh, :, m:a1])

            vslab = vpool.tile([128, NST * HD], bf16, tag="vslab")
            vview = vp[bh].rearrange("(t p) d -> p t d", p=128)
            vout = vslab[:].rearrange("p (t d) -> p t d", d=HD)
            for j, (a0, a1) in enumerate([(0, new_lo // 2), (new_lo // 2, new_lo),
                                          (new_hi, (new_hi + NST) // 2),
                                          ((new_hi + NST) // 2, NST)]):
                (nc.sync if j % 2 else nc.gpsimd).dma_start(
                    vout[:, a0:a1, :], vview[:, a0:a1, :])

            accv = aps.tile([128, Q], f32, tag="accv")
            dps = dps_pool.tile([1, Q], f32, tag="dps")
            for kt in range(NST):
                if new_lo <= kt < new_hi:
                    k_lhsT = qkT[(4 + h, b)][:, (kt - new_lo) * 128:
                                             (kt - new_lo + 1) * 128]
                else:
                    k_lhsT = kslab[:, kt * 128:(kt + 1) * 128]
                sps = scps.tile([128, Q], f32, tag="sps")
                nc.tensor.matmul(sps[:], k_lhsT, qkT[(h, b)][:],
                                 start=True, stop=True)
                e32 = epool.tile([128, Q], f32, tag="e32")
                nc.vector.tensor_add(e32[:], mask_slab[:, kt * Q:(kt + 1) * Q],
                                     sps[:])
                ebf = epool.tile([128, Q], bf16, tag="ebf")
                nc.scalar.activation(ebf[:], e32[:], AF.Exp)
                if new_lo <= kt < new_hi:
                    v_lhsT = v_new[b * 4 + (kt - new_lo)][:, h * HD:(h + 1) * HD]
                else:
                    v_lhsT = vslab[:, kt * HD:(kt + 1) * HD]
                nc.tensor.matmul(accv[:], v_lhsT, ebf[:],
                                 start=(kt == 0), stop=(kt == NST - 1))
                nc.tensor.matmul(dps[:], onesb_t[:], ebf[:],
                                 start=(kt == 0), stop=(kt == NST - 1))

            rec = dpool.tile([1, Q], f32, tag="rec")
            nc.vector.reciprocal(rec[:], dps[:])
            rec_r = dpool.tile([1, Q], f32r, tag="recr")
            nc.vector.tensor_copy(rec_r[:], rec[:])
            bps = bps_pool.tile([128, Q], f32, tag="bps")
            nc.tensor.matmul(bps[:], onesr_t[:], rec_r[:],
                             start=True, stop=True)
            bcs = dpool.tile([128, Q], f32, tag="bcs")
            nc.scalar.copy(bcs[:], bps[:])
            ot = persist.tile([128, Q], bf16, tag=f"outT_{bh}")
            nc.vector.tensor_mul(ot[:], bcs[:], accv[:])
            outT[bh] = ot

    # ---------------- phase 4: output projection partial -----------------
    with tc.tile_pool(name="wp4", bufs=1) as wp4, \
         tc.tile_pool(name="oev4", bufs=4) as oevp, \
         tc.tile_pool(name="pps4", bufs=3, space="PSUM") as pps4:
        wpt = {}
        for kt in range(HL):
            wpt[kt] = wp4.tile([128, D], bf16, tag=f"wp{kt}", name=f"wpt{kt}")
            (nc.sync if kt % 2 else nc.gpsimd).dma_start(
                wpt[kt][:], wp[kt * 128:(kt + 1) * 128, :])
        for b in range(B):
            for st in range(4):
                for ncn in range(8):
                    ops = pps4.tile([128, 512], f32, tag="ops")
                    for kt in range(HL):
                        nc.tensor.matmul(
                            ops[:],
                            outT[b * 4 + kt][:, st * 128:(st + 1) * 128],
                            wpt[kt][:, ncn * 512:(ncn + 1) * 512],
                            start=(kt == 0), stop=(kt == HL - 1))
                    oev = oevp.tile([128, 512], f32, tag="oev")
                    nc.scalar.copy(oev[:], ops[:])
                    (nc.sync if ncn % 2 else nc.gpsimd).dma_start(
                        outp[b * 512 + st * 128: b * 512 + (st + 1) * 128,
                             ncn * 512:(ncn + 1) * 512], oev[:])


# ------------------------- host side -------------------------

_PROGRAM_CACHE = {}
_FAST = True   # set by prepare_inputs; fast = zero mask and zero qkv bias


def build_program(step_tile: int, repeats: int = 1):
    if _FAST:
        return build_program_fast(step_tile, repeats)
    return build_program_generic(step_tile, repeats)


def _get_program(step_tile, repeats=1):
    key = (step_tile, repeats, _FAST)
    if key not in _PROGRAM_CACHE:
        _PROGRAM_CACHE[key] = build_program(step_tile, repeats)
    return _PROGRAM_CACHE[key]


def prepare_inputs(hidden_states, attention_mask, freqs, position_ids,
                   past_key, past_value, w_qkv, b_qkv, w_proj, b_proj,
                   current_step, layer_idx):
    """Shard + lay out inputs for the 8 cores. Returns (in_maps, step_tile)."""
    global _FAST
    hidden_states = np.asarray(hidden_states, dtype=np.float32)
    attention_mask = np.asarray(attention_mask, dtype=np.float32)
    freqs = np.asarray(freqs, dtype=np.float32)
    position_ids = np.asarray(position_ids)
    past_key = np.asarray(past_key, dtype=np.float32)
    past_value = np.asarray(past_value, dtype=np.float32)
    w_qkv = np.asarray(w_qkv, dtype=np.float32)
    b_qkv = np.asarray(b_qkv, dtype=np.float32)
    w_proj = np.asarray(w_proj, dtype=np.float32)
    current_step = int(current_step)
    scale = float(int(layer_idx) + 1)
    assert current_step % 128 == 0 and current_step + Q <= MAXLEN

    _FAST = (not attention_mask.any()) and (not b_qkv.any())
    if _FAST:
        return _prepare_fast(hidden_states, freqs, position_ids, past_key,
                             past_value, w_qkv, w_proj), current_step // 128
    return _prepare_generic(hidden_states, attention_mask, freqs,
                            position_ids, past_key, past_value, w_qkv, b_qkv,
                            w_proj, scale), current_step // 128


def _prepare_fast(hidden_states, freqs, position_ids, past_key, past_value,
                  w_qkv, w_proj):
    hTb = np.ascontiguousarray(
        hidden_states.reshape(BQ, D).T).astype(ml_dtypes.bfloat16)
    cos = np.cos(freqs)[position_ids]                                  # [B,Q,ROT]
    sin = np.sin(freqs)[position_ids]
    cosT = cos.transpose(0, 2, 1)                                      # [B,ROT,Q]
    sinm = np.ascontiguousarray(sin.transpose(0, 2, 1))
    sinm[:, :ROT // 2] *= -1.0
    cosb = np.ascontiguousarray(
        np.concatenate([cosT[0], cosT[1]], axis=1)).astype(ml_dtypes.bfloat16)
    sinmb = np.ascontiguousarray(
        np.concatenate([sinm[0], sinm[1]], axis=1)).astype(ml_dtypes.bfloat16)

    onesb = np.ones((128, 1), dtype=ml_dtypes.bfloat16)
    onesr = np.ones((1, 128), dtype=np.float32)

    in_maps = []
    for c in range(N_CORES):
        g0 = c * HL
        wq_c = np.ascontiguousarray(
            w_qkv[:, g0 * HD:(g0 + HL) * HD] * ALPHA).astype(ml_dtypes.bfloat16)
        wk_c = np.ascontiguousarray(
            w_qkv[:, D + g0 * HD: D + (g0 + HL) * HD]).astype(ml_dtypes.bfloat16)
        wv_c = np.ascontiguousarray(
            w_qkv[:, 2 * D + g0 * HD: 2 * D + (g0 + HL) * HD]
        ).astype(ml_dtypes.bfloat16)
        kTp_c = np.ascontiguousarray(
            past_key[:, g0:g0 + HL].transpose(0, 1, 3, 2)
        ).reshape(B * HL, HD, MAXLEN).astype(ml_dtypes.bfloat16)
        vsw_c = np.ascontiguousarray(
            past_value[:, g0:g0 + HL].reshape(B, HL, NST, 128, HD)
            .transpose(0, 1, 3, 2, 4)
        ).reshape(B * HL, 128, NST * HD).astype(ml_dtypes.bfloat16)
        wp_c = np.ascontiguousarray(w_proj[g0 * HD:(g0 + HL) * HD, :]).astype(
            ml_dtypes.bfloat16)
        in_maps.append(dict(
            hb=hTb, wq=wq_c, wk=wk_c, wv=wv_c, cosb=cosb, sinmb=sinmb,
            kTp=kTp_c, vsw=vsw_c, wp=wp_c, onesb=onesb, onesr=onesr))
    return in_maps


def _prepare_generic(hidden_states, attention_mask, freqs, position_ids,
                     past_key, past_value, w_qkv, b_qkv, w_proj, scale):
    hTf = np.ascontiguousarray(hidden_states.reshape(BQ, D).T)        # [D, BQ]
    hTb = hTf.astype(ml_dtypes.bfloat16)
    cos = np.cos(freqs)[position_ids]                                  # [B,Q,ROT]
    sin = np.sin(freqs)[position_ids]
    cosT = np.ascontiguousarray(cos.transpose(0, 2, 1))                # [B,ROT,Q]
    sinT = np.ascontiguousarray(sin.transpose(0, 2, 1))
    maskTf = np.ascontiguousarray(
        (attention_mask[:, 0] * scale).transpose(0, 2, 1))             # [B,MAXLEN,Q]

    R = np.zeros((ROT, ROT), dtype=np.float32)
    for i in range(ROT // 2):
        R[i, i + ROT // 2] = -1.0
        R[i + ROT // 2, i] = 1.0
    rmat = np.ascontiguousarray(R.T)

    ones = np.ones((1, BQ), dtype=np.float32)
    onesr = np.ones((1, 128), dtype=np.float32)
    onesb = np.ones((128, 1), dtype=ml_dtypes.bfloat16)
    onesbr = np.ones((1, 128), dtype=ml_dtypes.bfloat16)

    in_maps = []
    for c in range(N_CORES):
        g0 = c * HL
        wqk_c = np.empty((D, 8 * 128), dtype=np.float32)
        wqk_c[:, 0:512] = w_qkv[:, g0 * HD:(g0 + HL) * HD]
        wqk_c[:, 512:1024] = w_qkv[:, D + g0 * HD: D + (g0 + HL) * HD]
        wv_c = np.ascontiguousarray(
            w_qkv[:, 2 * D + g0 * HD: 2 * D + (g0 + HL) * HD]
        ).astype(ml_dtypes.bfloat16)
        bqk_c = np.empty((1, 8 * 128), dtype=np.float32)
        bqk_c[0, 0:512] = b_qkv[g0 * HD:(g0 + HL) * HD]
        bqk_c[0, 512:1024] = b_qkv[D + g0 * HD: D + (g0 + HL) * HD]
        bvb_c = np.ascontiguousarray(
            b_qkv[2 * D + g0 * HD: 2 * D + (g0 + HL) * HD]
        ).reshape(1, 512).astype(ml_dtypes.bfloat16)
        kTp_c = np.ascontiguousarray(
            past_key[:, g0:g0 + HL].transpose(0, 1, 3, 2)
        ).reshape(B * HL, HD, MAXLEN)
        vp_c = np.ascontiguousarray(past_value[:, g0:g0 + HL]).reshape(
            B * HL, MAXLEN, HD).astype(ml_dtypes.bfloat16)
        wp_c = np.ascontiguousarray(w_proj[g0 * HD:(g0 + HL) * HD, :]).astype(
            ml_dtypes.bfloat16)
        in_maps.append(dict(
            hT=hTf, hTb=hTb, wqk=wqk_c, wv=wv_c, bqk=bqk_c, bvb=bvb_c,
            ones=ones, onesr=onesr, onesb=onesb, onesbr=onesbr, rmat=rmat,
            kTp=kTp_c, vp=vp_c, maskT=maskTf, cosT=cosT, sinT=sinT, wp=wp_c))
    return in_maps


def assemble_output(results, b_proj):
    acc = np.zeros((BQ, D), dtype=np.float64)
    for r in results:
        acc += r["outp"].astype(np.float64)
    acc += np.asarray(b_proj, dtype=np.float64)[None, :]
    return acc.astype(np.float32).reshape(B, Q, D)


def kernel(**inputs):
    in_maps, step_tile = prepare_inputs(**inputs)
    nc = _get_program(step_tile)
    res = run_bass_kernel_spmd(nc, in_maps, core_ids=list(range(N_CORES)))
    return assemble_output(res.results, inputs["b_proj"])
